# revision 10
# baseline (speedup 1.0000x reference)
"""LoRA attention kernel for Trainium2, batch-sharded across 8 NeuronCores.

Strategy:
  - Data parallel: batch B=8 -> one batch element per core.
  - LoRA factors are folded into Wqkv on the host (exact algebra, float64):
        q' = q @ (I + Aq Bq)  ==>  Wq' = (I + Aq Bq).T @ Wq   (per head)
  - All matmuls run as float32r (fp32 with 11-bit mantissa, full PE rate).
  - q,k are produced transposed ([head_dim, tokens]) directly from x^T so the
    score matmuls need no on-chip transposes. v is produced in natural layout
    with an extra all-ones column per head, so the attention-value matmul
    accumulates the softmax denominators for free in its last output row.
  - Scores are computed transposed, s[k, q]; softmax normalization is applied
    to the (small) attention output instead of the score matrix.
  - The output projection consumes the transposed attention output directly
    and produces y in natural layout; bias is fused into the PSUM drain.
  - Every matmul moving operand is a standalone tile with pitch == N
    (strided moving operands run at half rate on the PE).
"""
import numpy as np

import concourse.bass as bass
import concourse.bacc as bacc
import concourse.mybir as mybir
import concourse.tile as tile
from concourse.bass_utils import run_bass_kernel_spmd

F32 = mybir.dt.float32
F32R = mybir.dt.float32r
EXP = mybir.ActivationFunctionType.Exp

B, N, C, H, HD = 8, 1024, 768, 12, 64
CT = C // 128           # 6 contraction tiles over C
TT = N // 128           # 8 token tiles
QC = N // 512           # 2 query chunks of 512
KT = N // 128           # 8 key tiles of 128
EC = 2                  # output-projection feature chunks of 384
SCALE = HD ** -0.5
N_CORES = 8

_NC_CACHE = None


def _build():
    nc = bacc.Bacc(None, target_bir_lowering=False)

    xT = nc.dram_tensor("xT", [C, N], F32R, kind="ExternalInput")
    wqk = nc.dram_tensor("wqk", [H, CT, 128, 128], F32R, kind="ExternalInput")
    wv = nc.dram_tensor("wv", [CT, 128, C], F32R, kind="ExternalInput")
    wpt = nc.dram_tensor("wpt", [CT, 128, C], F32R, kind="ExternalInput")
    bias = nc.dram_tensor("bias", [1, C], F32, kind="ExternalInput")
    y = nc.dram_tensor("y", [N, C], F32, kind="ExternalOutput")

    from contextlib import ExitStack
    with tile.TileContext(nc) as tc:
        with ExitStack() as ctx:
            pool = lambda name, bufs, **kw: ctx.enter_context(
                tc.tile_pool(name=name, bufs=bufs, **kw))
            xt_pool = pool("xt", 2 * CT)
            wqk_pool = pool("wqkp", 2 * CT)
            w384_pool = pool("w384", 2 * CT)      # wv then wpt (disjoint phases)
            vaug_pool = pool("vaug", TT)
            st_pool = pool("stp", 6)
            kt_pool = pool("ktp", 6)
            exp_pool = pool("expp", 8)
            avs_pool = pool("avsp", 4)
            iv_pool = pool("ivp", 3)
            bc_pool = pool("bcp", 3)
            ost_pool = pool("ostp", 3)
            out_pool = pool("outp", CT)
            y_pool = pool("yp", 3)
            cst_pool = pool("cst", 1)
            proj_ps = pool("proj_ps", 2, space="PSUM")
            sc_ps = pool("sc_ps", 3, space="PSUM")
            av_ps = pool("av_ps", 3, space="PSUM")

            # ---- PE warm-up: dummy matmuls bridge the DMA lead-in so the
            # HAM clock gate opens before real work arrives -----------------
            wuf = cst_pool.tile([128, 512], F32, tag="wuf")
            nc.vector.memset(wuf, 0.0)
            wur = cst_pool.tile([128, 512], F32R, tag="wur")
            nc.vector.tensor_copy(wur, wuf)

            def warmup(n, label):
                for i in range(n):
                    wps = proj_ps.tile([128, 512], F32, tag="mmps",
                                       name=f"wu{label}_{i}")
                    nc.tensor.matmul(wps, wur[:, 0:128], wur,
                                     start=True, stop=True)

            warmup(14, "a")

            # ---- loads ---------------------------------------------------
            def load_wqk(h):
                wts = []
                for c in range(CT):
                    wt = wqk_pool.tile([128, 128], F32R, tag="wqk",
                                       name=f"wqk{h}_{c}")
                    nc.sync.dma_start(out=wt, in_=wqk[h, c, :, :])
                    wts.append(wt)
                return wts

            wts0 = load_wqk(0)

            # x^T in 12 standalone [128, 512] tiles (contiguous pitch)
            xt = [[None] * QC for _ in range(CT)]
            for c in range(CT):
                for qc in range(QC):
                    t = xt_pool.tile([128, 512], F32R, tag="xt",
                                     name=f"xt{c}_{qc}")
                    nc.sync.dma_start(
                        out=t, in_=xT[c * 128:(c + 1) * 128,
                                      qc * 512:(qc + 1) * 512])
                    xt[c][qc] = t

            bias_bc = cst_pool.tile([128, C], F32, tag="biasbc")
            nc.sync.dma_start(out=bias_bc, in_=bias[:, :].to_broadcast([128, C]))
            ones12 = cst_pool.tile([128, H], F32, tag="ones12")
            nc.vector.memset(ones12, 1.0)

            wvt = [[None] * 2 for _ in range(CT)]
            for c in range(CT):
                for half in range(2):
                    t = w384_pool.tile([128, 384], F32R, tag="w384",
                                       name=f"wv{c}_{half}")
                    nc.sync.dma_start(
                        out=t, in_=wv[c, :, half * 384:(half + 1) * 384])
                    wvt[c][half] = t

            # ---- per-head q/k projection ---------------------------------
            def qk_project(h, wts):
                """q (rows 0-63) and k (rows 64-127), transposed layout.
                Returns ([stA, stB], [ktA, ktB]) per 512-token chunk."""
                sts, kts = [], []
                for qc in range(QC):
                    st = st_pool.tile([128, 512], F32R, tag="st",
                                      name=f"st{h}_{qc}")
                    pqk = proj_ps.tile([128, 512], F32, tag="mmps",
                                       name=f"pqk{h}_{qc}")
                    for c in range(CT):
                        nc.tensor.matmul(
                            pqk, wts[c], xt[c][qc],
                            start=(c == 0), stop=(c == CT - 1),
                        )
                    nc.vector.tensor_copy(st, pqk)
                    # move k rows to a base-0 tile (partition shift via DMA)
                    kt_t = kt_pool.tile([64, 512], F32R, tag="kt",
                                        name=f"kt{h}_{qc}")
                    nc.sync.dma_start(out=kt_t, in_=st[64:128, :])
                    sts.append(st)
                    kts.append(kt_t)
                return sts, kts

            head0 = qk_project(0, wts0)

            # ---- v_aug[tt] = [v | 1] per head, natural layout ------------
            vaug = []
            for tt in range(TT):
                va = vaug_pool.tile([128, H * 65], F32R, tag="vaug",
                                    name=f"vaug{tt}")
                for half in range(2):
                    pv = proj_ps.tile([128, 384], F32, tag="mmps",
                                      name=f"pv{tt}_{half}")
                    for c in range(CT):
                        nc.tensor.matmul(
                            pv,
                            xt[c][tt // 4][:, (tt % 4) * 128:(tt % 4 + 1) * 128],
                            wvt[c][half],
                            start=(c == 0), stop=(c == CT - 1),
                        )
                    dst = bass.AP(tensor=va.tensor,
                                  offset=va.offset + half * 6 * 65,
                                  ap=[va.ap[0], [65, 6], [1, 64]])
                    nc.vector.tensor_copy(dst, pv)
                ones_ap = bass.AP(tensor=va.tensor, offset=va.offset + 64,
                                  ap=[va.ap[0], [65, H]])
                nc.vector.tensor_copy(ones_ap, ones12)
                vaug.append(va)

            # ---- output accumulator tiles (c-major, [128, N]) ------------
            outT = [out_pool.tile([128, N], F32R, tag="outT", name=f"outT{i}")
                    for i in range(CT)]

            # ---- per-head attention --------------------------------------
            wptt = None
            for h in range(H):
                sts, kts = head0 if h == 0 else qk_project(h, load_wqk(h))

                for qc in range(QC):
                    q_sl = sts[qc][0:64, :]
                    av = av_ps.tile([65, 512], F32, tag="av",
                                    name=f"av{h}_{qc}")
                    for kt in range(KT):
                        ps_s = sc_ps.tile([128, 512], F32, tag="sc",
                                          name=f"sc{h}_{qc}_{kt}")
                        nc.tensor.matmul(
                            ps_s,
                            kts[kt // 4][:, (kt % 4) * 128:(kt % 4 + 1) * 128],
                            q_sl, start=True, stop=True,
                        )
                        et = exp_pool.tile([128, 512], F32R, tag="exp",
                                           name=f"exp{h}_{qc}_{kt}")
                        nc.scalar.activation(out=et, in_=ps_s, func=EXP,
                                             scale=SCALE)
                        nc.tensor.matmul(
                            av, vaug[kt][:, h * 65:(h + 1) * 65], et,
                            start=(kt == 0), stop=(kt == KT - 1),
                        )
                    # drain the whole psum quickly to release the bank; the
                    # normalization then runs off the PE critical path
                    avs = avs_pool.tile([65, 512], F32, tag="avs",
                                        name=f"avs{h}_{qc}")
                    nc.vector.tensor_copy(avs, av)
                    # row 64 of avs = softmax denominators for this q chunk.
                    # DMA-shift them to partition 0, then fast-reciprocal and
                    # broadcast (both require base partition 0).
                    sm0 = iv_pool.tile([1, 512], F32, tag="sm0",
                                       name=f"sm0{h}_{qc}")
                    nc.sync.dma_start(out=sm0, in_=avs[64:65, :])
                    iv0 = iv_pool.tile([1, 512], F32, tag="iv0",
                                       name=f"iv0{h}_{qc}")
                    nc.vector.reciprocal_approx_fast(out=iv0, in_=sm0)
                    bc = bc_pool.tile([64, 512], F32, tag="bc",
                                      name=f"bc{h}_{qc}")
                    nc.gpsimd.partition_broadcast(bc, iv0)

                    ct_i = h // 2
                    if h % 2 == 0:
                        nc.vector.tensor_mul(
                            outT[ct_i][0:64, qc * 512:(qc + 1) * 512],
                            avs[0:64, :], bc)
                    else:
                        ost = ost_pool.tile([64, 512], F32R, tag="ost",
                                            name=f"ost{h}_{qc}")
                        nc.vector.tensor_mul(ost, avs[0:64, :], bc)
                        nc.sync.dma_start(
                            out=outT[ct_i][64:128, qc * 512:(qc + 1) * 512],
                            in_=ost)

                if h == 5:
                    # prefetch output-projection weights mid-flight
                    wptt = [[None] * EC for _ in range(CT)]
                    for c in range(CT):
                        for ec in range(EC):
                            t = w384_pool.tile([128, 384], F32R, tag="w384",
                                               name=f"wpt{c}_{ec}")
                            nc.sync.dma_start(
                                out=t,
                                in_=wpt[c, :, ec * 384:(ec + 1) * 384])
                            wptt[c][ec] = t

            # ---- output projection ---------------------------------------
            warmup(4, "b")
            for tt in range(TT):
                ysb = y_pool.tile([128, C], F32, tag="y", name=f"y{tt}")
                for ec in range(EC):
                    py = proj_ps.tile([128, 384], F32, tag="mmps",
                                      name=f"py{tt}_{ec}")
                    for c in range(CT):
                        nc.tensor.matmul(
                            py,
                            outT[c][:, tt * 128:(tt + 1) * 128],
                            wptt[c][ec],
                            start=(c == 0), stop=(c == CT - 1),
                        )
                    nc.vector.tensor_add(ysb[:, ec * 384:(ec + 1) * 384], py,
                                         bias_bc[:, ec * 384:(ec + 1) * 384])
                nc.sync.dma_start(out=y[tt * 128:(tt + 1) * 128, :], in_=ysb)

    nc.finalize()
    return nc


def _get_nc():
    global _NC_CACHE
    if _NC_CACHE is None:
        _NC_CACHE = _build()
    return _NC_CACHE


def _host_prep(x, Wqkv, Wproj, bproj, Aq, Bq, Av, Bv):
    """Fold LoRA into the weights and lay everything out for the kernel."""
    W = Wqkv.astype(np.float64)
    Wq = W[0:C].reshape(H, HD, C)
    Wk = W[C:2 * C].reshape(H, HD, C)
    Wv_ = W[2 * C:3 * C].reshape(H, HD, C)
    ABq = Aq.astype(np.float64) @ Bq.astype(np.float64)   # [HD, HD]
    ABv = Av.astype(np.float64) @ Bv.astype(np.float64)
    Wq = Wq + np.einsum('ed,hec->hdc', ABq, Wq)           # (I+AB).T @ Wq per head
    Wv_ = Wv_ + np.einsum('ed,hec->hdc', ABv, Wv_)

    # wqk[h, c] = [K=c-rows(128), M = q_h cols(64) ++ k_h cols(64)]
    wqk = np.empty((H, CT, 128, 128), np.float32)
    for h in range(H):
        for c in range(CT):
            cs = slice(c * 128, (c + 1) * 128)
            wqk[h, c, :, 0:64] = Wq[h][:, cs].T.astype(np.float32)
            wqk[h, c, :, 64:128] = Wk[h][:, cs].astype(np.float32).T

    # wv[c] = [K=c-rows(128), all 768 v output features]
    WvT = Wv_.reshape(C, C).T.astype(np.float32)          # [c_in, v_out]
    wv = np.ascontiguousarray(WvT.reshape(CT, 128, C))

    # wpt[c] = Wproj.T c-tiles: [K=c(128), e(768)]
    WpT = Wproj.astype(np.float32).T                      # [c, e]
    wpt = np.ascontiguousarray(WpT.reshape(CT, 128, C))

    bias = bproj.astype(np.float32).reshape(1, C)

    per_core = []
    for b in range(B):
        xTb = np.ascontiguousarray(x[b].astype(np.float32).T)   # [C, N]
        per_core.append({"xT": xTb, "wqk": wqk, "wv": wv, "wpt": wpt,
                         "bias": bias})
    return per_core


def kernel(x, Wqkv, Wproj, bproj, Aq, Bq, Av, Bv, _trace=False):
    x = np.asarray(x)
    in_maps = _host_prep(np.asarray(x), np.asarray(Wqkv), np.asarray(Wproj),
                         np.asarray(bproj), np.asarray(Aq), np.asarray(Bq),
                         np.asarray(Av), np.asarray(Bv))
    nc = _get_nc()
    res = run_bass_kernel_spmd(nc, in_maps, core_ids=list(range(N_CORES)),
                               trace=_trace)
    out = np.stack([res.results[b]["y"] for b in range(B)], axis=0)
    if _trace:
        kernel._last_result = res
    return out.astype(np.float32)


# revision 11
# speedup vs baseline: 1.3356x; 1.3356x over previous
"""LoRA attention kernel for Trainium2, batch-sharded across 8 NeuronCores.

Strategy:
  - Data parallel: batch B=8 -> one batch element per core.
  - LoRA factors are folded into Wqkv on the host (exact algebra, float64):
        q' = q @ (I + Aq Bq)  ==>  Wq' = (I + Aq Bq).T @ Wq   (per head)
  - All matmuls run as float32r (fp32 with 11-bit mantissa, full PE rate).
  - q,k are produced transposed ([head_dim, tokens]) directly from x^T so the
    score matmuls need no on-chip transposes. v is produced in natural layout
    with an extra all-ones column per head, so the attention-value matmul
    accumulates the softmax denominators for free in its last output row.
  - Scores are computed transposed, s[k, q]; softmax normalization is applied
    to the (small) attention output instead of the score matrix.
  - The output projection consumes the transposed attention output directly
    and produces y in natural layout; bias is fused into the PSUM drain.
  - Every matmul moving operand is a standalone tile with pitch == N
    (strided moving operands run at half rate on the PE).
"""
import numpy as np

import concourse.bass as bass
import concourse.bacc as bacc
import concourse.mybir as mybir
import concourse.tile as tile
from concourse.bass_utils import run_bass_kernel_spmd

F32 = mybir.dt.float32
F32R = mybir.dt.float32r
EXP = mybir.ActivationFunctionType.Exp

B, N, C, H, HD = 8, 1024, 768, 12, 64
CT = C // 128           # 6 contraction tiles over C
TT = N // 128           # 8 token tiles
QC = N // 512           # 2 query chunks of 512
KT = N // 128           # 8 key tiles of 128
EC = 2                  # output-projection feature chunks of 384
SCALE = HD ** -0.5
N_CORES = 8

_NC_CACHE = None


def _build():
    nc = bacc.Bacc(None, target_bir_lowering=False)

    xT = nc.dram_tensor("xT", [C, N], F32R, kind="ExternalInput")
    wqk = nc.dram_tensor("wqk", [H, CT, 128, 128], F32R, kind="ExternalInput")
    wv = nc.dram_tensor("wv", [CT, 128, C], F32R, kind="ExternalInput")
    wpt = nc.dram_tensor("wpt", [CT, 128, C], F32R, kind="ExternalInput")
    bias = nc.dram_tensor("bias", [1, C], F32, kind="ExternalInput")
    y = nc.dram_tensor("y", [N, C], F32, kind="ExternalOutput")

    from contextlib import ExitStack
    with tile.TileContext(nc) as tc:
        with ExitStack() as ctx:
            pool = lambda name, bufs, **kw: ctx.enter_context(
                tc.tile_pool(name=name, bufs=bufs, **kw))
            xt_pool = pool("xt", 2 * CT)
            wqk_pool = pool("wqkp", 2 * CT)
            w384_pool = pool("w384", 2 * CT)      # wv then wpt (disjoint phases)
            vaug_pool = pool("vaug", TT)
            st_pool = pool("stp", 6)
            kt_pool = pool("ktp", 6)
            exp_pool = pool("expp", 8)
            avs_pool = pool("avsp", 4)
            iv_pool = pool("ivp", 3)
            bc_pool = pool("bcp", 3)
            ost_pool = pool("ostp", 3)
            out_pool = pool("outp", CT)
            y_pool = pool("yp", 3)
            cst_pool = pool("cst", 1)
            proj_ps = pool("proj_ps", 2, space="PSUM")
            sc_ps = pool("sc_ps", 3, space="PSUM")
            av_ps = pool("av_ps", 3, space="PSUM")

            # ---- PE warm-up: dummy matmuls bridge the DMA lead-in so the
            # HAM clock gate opens before real work arrives -----------------
            wuf = cst_pool.tile([128, 512], F32, tag="wuf")
            nc.vector.memset(wuf, 0.0)
            wur = cst_pool.tile([128, 512], F32R, tag="wur")
            nc.vector.tensor_copy(wur, wuf)

            def warmup(n, label):
                for i in range(n):
                    wps = proj_ps.tile([128, 512], F32, tag="mmps",
                                       name=f"wu{label}_{i}")
                    nc.tensor.matmul(wps, wur[:, 0:128], wur,
                                     start=True, stop=True)

            warmup(14, "a")

            # ---- loads ---------------------------------------------------
            def load_wqk(h):
                wts = []
                for c in range(CT):
                    wt = wqk_pool.tile([128, 128], F32R, tag="wqk",
                                       name=f"wqk{h}_{c}")
                    nc.sync.dma_start(out=wt, in_=wqk[h, c, :, :])
                    wts.append(wt)
                return wts

            wts0 = load_wqk(0)

            # x^T in 12 standalone [128, 512] tiles (contiguous pitch)
            xt = [[None] * QC for _ in range(CT)]
            for c in range(CT):
                for qc in range(QC):
                    t = xt_pool.tile([128, 512], F32R, tag="xt",
                                     name=f"xt{c}_{qc}")
                    nc.sync.dma_start(
                        out=t, in_=xT[c * 128:(c + 1) * 128,
                                      qc * 512:(qc + 1) * 512])
                    xt[c][qc] = t

            bias_bc = cst_pool.tile([128, C], F32, tag="biasbc")
            nc.sync.dma_start(out=bias_bc, in_=bias[:, :].to_broadcast([128, C]))
            ones12 = cst_pool.tile([128, H], F32, tag="ones12")
            nc.vector.memset(ones12, 1.0)

            wvt = [[None] * 2 for _ in range(CT)]
            for c in range(CT):
                for half in range(2):
                    t = w384_pool.tile([128, 384], F32R, tag="w384",
                                       name=f"wv{c}_{half}")
                    nc.sync.dma_start(
                        out=t, in_=wv[c, :, half * 384:(half + 1) * 384])
                    wvt[c][half] = t

            # ---- per-head q/k projection ---------------------------------
            def qk_project(h, wts):
                """q (rows 0-63) and k (rows 64-127), transposed layout.
                Returns ([stA, stB], [ktA, ktB]) per 512-token chunk."""
                sts, kts = [], []
                for qc in range(QC):
                    st = st_pool.tile([128, 512], F32R, tag="st",
                                      name=f"st{h}_{qc}")
                    pqk = proj_ps.tile([128, 512], F32, tag="mmps",
                                       name=f"pqk{h}_{qc}")
                    for c in range(CT):
                        nc.tensor.matmul(
                            pqk, wts[c], xt[c][qc],
                            start=(c == 0), stop=(c == CT - 1),
                        )
                    nc.vector.tensor_copy(st, pqk)
                    # move k rows to the top of a base-0 tile (partition shift
                    # via DMA) and zero rows 64-127 so the score matmuls can
                    # run with K=128 (uniform PE tile config; zeros are exact)
                    kt_t = kt_pool.tile([128, 512], F32R, tag="kt",
                                        name=f"kt{h}_{qc}")
                    nc.sync.dma_start(out=kt_t[0:64, :], in_=st[64:128, :])
                    nc.vector.tensor_copy(kt_t[64:128, :], wur[64:128, :])
                    sts.append(st)
                    kts.append(kt_t)
                return sts, kts

            head0 = qk_project(0, wts0)

            # ---- v_aug[tt] = [v | 1] per head, natural layout ------------
            vaug = []
            for tt in range(TT):
                va = vaug_pool.tile([128, (H - 1) * 65 + 128], F32R,
                                    tag="vaug", name=f"vaug{tt}")
                for half in range(2):
                    pv = proj_ps.tile([128, 384], F32, tag="mmps",
                                      name=f"pv{tt}_{half}")
                    for c in range(CT):
                        nc.tensor.matmul(
                            pv,
                            xt[c][tt // 4][:, (tt % 4) * 128:(tt % 4 + 1) * 128],
                            wvt[c][half],
                            start=(c == 0), stop=(c == CT - 1),
                        )
                    dst = bass.AP(tensor=va.tensor,
                                  offset=va.offset + half * 6 * 65,
                                  ap=[va.ap[0], [65, 6], [1, 64]])
                    nc.vector.tensor_copy(dst, pv)
                ones_ap = bass.AP(tensor=va.tensor, offset=va.offset + 64,
                                  ap=[va.ap[0], [65, H]])
                nc.vector.tensor_copy(ones_ap, ones12)
                # zero the tail cols so the widened av lhsT reads no garbage
                nc.vector.tensor_copy(va[:, H * 65:], wur[:, 0:(H - 1) * 65 + 128 - H * 65])
                vaug.append(va)

            # ---- output accumulator tiles (c-major, [128, N]) ------------
            outT = [out_pool.tile([128, N], F32R, tag="outT", name=f"outT{i}")
                    for i in range(CT)]

            # ---- per-head attention --------------------------------------
            wptt = None
            for h in range(H):
                sts, kts = head0 if h == 0 else qk_project(h, load_wqk(h))

                for qc in range(QC):
                    av = av_ps.tile([128, 512], F32, tag="av",
                                    name=f"av{h}_{qc}")
                    for kt in range(KT):
                        ps_s = sc_ps.tile([128, 512], F32, tag="sc",
                                          name=f"sc{h}_{qc}_{kt}")
                        nc.tensor.matmul(
                            ps_s,
                            kts[kt // 4][:, (kt % 4) * 128:(kt % 4 + 1) * 128],
                            sts[qc], start=True, stop=True,
                        )
                        et = exp_pool.tile([128, 512], F32R, tag="exp",
                                           name=f"exp{h}_{qc}_{kt}")
                        nc.scalar.activation(out=et, in_=ps_s, func=EXP,
                                             scale=SCALE)
                        nc.tensor.matmul(
                            av, vaug[kt][:, h * 65:h * 65 + 128], et,
                            start=(kt == 0), stop=(kt == KT - 1),
                        )
                    # drain the whole psum quickly to release the bank; the
                    # normalization then runs off the PE critical path
                    avs = avs_pool.tile([65, 512], F32, tag="avs",
                                        name=f"avs{h}_{qc}")
                    nc.vector.tensor_copy(avs, av[0:65, :])
                    # row 64 of avs = softmax denominators for this q chunk.
                    # DMA-shift them to partition 0, then fast-reciprocal and
                    # broadcast (both require base partition 0).
                    sm0 = iv_pool.tile([1, 512], F32, tag="sm0",
                                       name=f"sm0{h}_{qc}")
                    nc.sync.dma_start(out=sm0, in_=avs[64:65, :])
                    iv0 = iv_pool.tile([1, 512], F32, tag="iv0",
                                       name=f"iv0{h}_{qc}")
                    nc.vector.reciprocal_approx_fast(out=iv0, in_=sm0)
                    bc = bc_pool.tile([64, 512], F32, tag="bc",
                                      name=f"bc{h}_{qc}")
                    nc.gpsimd.partition_broadcast(bc, iv0)

                    ct_i = h // 2
                    if h % 2 == 0:
                        nc.vector.tensor_mul(
                            outT[ct_i][0:64, qc * 512:(qc + 1) * 512],
                            avs[0:64, :], bc)
                    else:
                        ost = ost_pool.tile([64, 512], F32R, tag="ost",
                                            name=f"ost{h}_{qc}")
                        nc.vector.tensor_mul(ost, avs[0:64, :], bc)
                        nc.sync.dma_start(
                            out=outT[ct_i][64:128, qc * 512:(qc + 1) * 512],
                            in_=ost)

                if h == 5:
                    # prefetch output-projection weights mid-flight
                    wptt = [[None] * EC for _ in range(CT)]
                    for c in range(CT):
                        for ec in range(EC):
                            t = w384_pool.tile([128, 384], F32R, tag="w384",
                                               name=f"wpt{c}_{ec}")
                            nc.sync.dma_start(
                                out=t,
                                in_=wpt[c, :, ec * 384:(ec + 1) * 384])
                            wptt[c][ec] = t

            # ---- output projection ---------------------------------------
            warmup(4, "b")
            for tt in range(TT):
                ysb = y_pool.tile([128, C], F32, tag="y", name=f"y{tt}")
                for ec in range(EC):
                    py = proj_ps.tile([128, 384], F32, tag="mmps",
                                      name=f"py{tt}_{ec}")
                    for c in range(CT):
                        nc.tensor.matmul(
                            py,
                            outT[c][:, tt * 128:(tt + 1) * 128],
                            wptt[c][ec],
                            start=(c == 0), stop=(c == CT - 1),
                        )
                    nc.vector.tensor_add(ysb[:, ec * 384:(ec + 1) * 384], py,
                                         bias_bc[:, ec * 384:(ec + 1) * 384])
                nc.sync.dma_start(out=y[tt * 128:(tt + 1) * 128, :], in_=ysb)

    nc.finalize()
    return nc


def _get_nc():
    global _NC_CACHE
    if _NC_CACHE is None:
        _NC_CACHE = _build()
    return _NC_CACHE


def _host_prep(x, Wqkv, Wproj, bproj, Aq, Bq, Av, Bv):
    """Fold LoRA into the weights and lay everything out for the kernel."""
    W = Wqkv.astype(np.float64)
    Wq = W[0:C].reshape(H, HD, C)
    Wk = W[C:2 * C].reshape(H, HD, C)
    Wv_ = W[2 * C:3 * C].reshape(H, HD, C)
    ABq = Aq.astype(np.float64) @ Bq.astype(np.float64)   # [HD, HD]
    ABv = Av.astype(np.float64) @ Bv.astype(np.float64)
    Wq = Wq + np.einsum('ed,hec->hdc', ABq, Wq)           # (I+AB).T @ Wq per head
    Wv_ = Wv_ + np.einsum('ed,hec->hdc', ABv, Wv_)

    # wqk[h, c] = [K=c-rows(128), M = q_h cols(64) ++ k_h cols(64)]
    wqk = np.empty((H, CT, 128, 128), np.float32)
    for h in range(H):
        for c in range(CT):
            cs = slice(c * 128, (c + 1) * 128)
            wqk[h, c, :, 0:64] = Wq[h][:, cs].T.astype(np.float32)
            wqk[h, c, :, 64:128] = Wk[h][:, cs].astype(np.float32).T

    # wv[c] = [K=c-rows(128), all 768 v output features]
    WvT = Wv_.reshape(C, C).T.astype(np.float32)          # [c_in, v_out]
    wv = np.ascontiguousarray(WvT.reshape(CT, 128, C))

    # wpt[c] = Wproj.T c-tiles: [K=c(128), e(768)]
    WpT = Wproj.astype(np.float32).T                      # [c, e]
    wpt = np.ascontiguousarray(WpT.reshape(CT, 128, C))

    bias = bproj.astype(np.float32).reshape(1, C)

    per_core = []
    for b in range(B):
        xTb = np.ascontiguousarray(x[b].astype(np.float32).T)   # [C, N]
        per_core.append({"xT": xTb, "wqk": wqk, "wv": wv, "wpt": wpt,
                         "bias": bias})
    return per_core


def kernel(x, Wqkv, Wproj, bproj, Aq, Bq, Av, Bv, _trace=False):
    x = np.asarray(x)
    in_maps = _host_prep(np.asarray(x), np.asarray(Wqkv), np.asarray(Wproj),
                         np.asarray(bproj), np.asarray(Aq), np.asarray(Bq),
                         np.asarray(Av), np.asarray(Bv))
    nc = _get_nc()
    res = run_bass_kernel_spmd(nc, in_maps, core_ids=list(range(N_CORES)),
                               trace=_trace)
    out = np.stack([res.results[b]["y"] for b in range(B)], axis=0)
    if _trace:
        kernel._last_result = res
    return out.astype(np.float32)
